# revision 5
# baseline (speedup 1.0000x reference)
"""KGAttentionLayer Trainium2 kernel (v7: block-0 v-proj interleaved into attention, bf16 output).

Sharding: 8 cores = (batch 2) x (query-block 4). Core c handles batch
b=c//4, query rows [j*512, (j+1)*512) of that batch (j=c%4); k/v/kg
projections duplicated within each 4-core batch group (no collectives).

v4 on top of v3/G1:
  - block b+1's projections are emitted as a generator whose psum-groups
    are interleaved into block b's attention stream: the attention inner
    loop is ACT(exp)-bound by ~260ns per slab group, and the interleaved
    projection matmuls keep the PE busy through those gaps (also keeps
    the HAM clock at 2.4 GHz).
  - xT is loaded as four [128,8,512] tiles so the first k-projection
    matmul starts after ~1MB of DMA instead of 4MB.
  - exp batched 2 chunks per ACT instruction ([128,1024] PSUM slabs).
  - softmax epilogue: numerator copy on DVE (PSUM partitions 0:64);
    the denominator row goes through ACT -- a DVE read at PSUM
    partition 64 silently returns garbage on HW.
  - tail out-proj/gate weights double-buffered one g ahead; gate
    arithmetic fused with scalar_tensor_tensor.

Layouts as v2: xT rolled so the core's queries are cols [0,512);
scoresT chunks via lhsT=kT[64,m128], rhs=qT[64,l512]; vslab carries a
ones column so attn@v emits numerator rows 0..63 + denominator row 64.
"""

import sys

sys.path.insert(0, "/opt/trn_rl_repo")

import numpy as np

import concourse.bass as bass
import concourse.mybir as mybir
import concourse.tile as tile
from concourse import bacc
from concourse.bass_utils import run_bass_kernel_spmd

F32 = mybir.dt.float32
BF16 = mybir.dt.bfloat16
F8 = mybir.dt.float8e4
DR = mybir.MatmulPerfMode.DoubleRow
AF = mybir.ActivationFunctionType
OP = mybir.AluOpType

D = 1024
H = 16
HD = 64
B = 2
L = 2048
E = 256
LBLK = 512
M = L + E
NMC = M // 128
N_CORES = 8

_CACHE = {}


def _build():
    nc = bacc.Bacc("TRN2", target_bir_lowering=False, debug=False,
                   num_devices=N_CORES)

    dram = {}

    def din(name, shape, dt=BF16):
        dram[name] = nc.dram_tensor(name, shape, dt, kind="ExternalInput")
        return dram[name]

    xT = din("xT", [D, L], F8)
    xres = din("xres", [128, 8, LBLK], F32)
    kgT = din("kgT", [D, E], F8)
    WqT = din("WqT", [D, D], F8)
    WkT = din("WkT", [D, D], F8)
    WkkT = din("WkkT", [D, D], F8)
    WvT = din("WvT", [D, D], F8)
    WkvT = din("WkvT", [D, D], F8)
    WoT = din("WoT", [D, D], F8)
    WgT = din("WgT", [D, D], F8)
    bq = din("bq", [128, 8], F32)
    bk = din("bk", [128, 8], F32)
    bkk = din("bkk", [128, 8], F32)
    bo = din("bo", [128, 8], F32)
    bge = din("bge", [128, 8], F32)
    bvb = din("bvb", [128, D], F32)
    bkvb = din("bkvb", [128, D], F32)

    OUTT = nc.dram_tensor("OUTT", [D, LBLK], BF16, kind="ExternalOutput")

    def w8(pool, W, g, tag="w8"):
        # DoubleRow stationary: [p, t, two, d] -- chunk pair (2t, 2t+1)
        t = pool.tile([128, 4, 2, 128], F8, tag=tag, name=f"{tag}_{W.name}_{g}")
        nc.sync.dma_start(
            t[:], W.ap().rearrange("(t two p) d -> p t two d", p=128, two=2)
            [:, :, :, g * 128:(g + 1) * 128])
        return t

    from contextlib import ExitStack

    with tile.TileContext(nc) as tc, ExitStack() as ctx:
        persist = ctx.enter_context(tc.tile_pool(name="persist", bufs=1))
        wpool = ctx.enter_context(tc.tile_pool(name="wpool", bufs=2))
        wqk = ctx.enter_context(tc.tile_pool(name="wqk", bufs=6))
        wog = ctx.enter_context(tc.tile_pool(name="wog", bufs=4))
        spool = ctx.enter_context(tc.tile_pool(name="spool", bufs=3))
        epool = ctx.enter_context(tc.tile_pool(name="epool", bufs=3))
        blkpool = ctx.enter_context(tc.tile_pool(name="blkpool", bufs=2))
        psProj = ctx.enter_context(tc.tile_pool(name="psProj", bufs=2,
                                                space="PSUM"))
        psSlab = ctx.enter_context(tc.tile_pool(name="psSlab", bufs=2,
                                                space="PSUM"))
        psAv = ctx.enter_context(tc.tile_pool(name="psAv", bufs=2,
                                              space="PSUM"))

        # ---- resident loads; x in 4 pieces so compute starts early ----
        xts4 = [persist.tile([128, 4, 2, 512], F8, tag=f"xts{lc}",
                             name=f"xts{lc}")
                for lc in range(4)]
        nc.sync.dma_start(
            xts4[0][:], xT.ap().rearrange("(t two p) l -> p t two l", p=128, two=2)
            [:, :, :, 0:512])
        biases = {}
        for nm in ("bq", "bk", "bkk", "bo", "bge"):
            t = persist.tile([128, 8], F32, tag=nm, name=nm + "_sb")
            biases[nm] = t
        nc.sync.dma_start(biases["bq"][:], dram["bq"].ap())
        nc.sync.dma_start(biases["bk"][:], dram["bk"].ap())

        def load_bulk():
            """Issued after block-0's weight DMAs so they don't queue
            behind 3MB of x."""
            for lc in range(1, 4):
                nc.sync.dma_start(
                    xts4[lc][:],
                    xT.ap().rearrange("(t two p) l -> p t two l", p=128, two=2)
                    [:, :, :, lc * 512:(lc + 1) * 512])
            nc.sync.dma_start(
                kgts[:], kgT.ap().rearrange("(t two p) e -> p t two e", p=128, two=2))
            for nm in ("bkk", "bo", "bge"):
                nc.sync.dma_start(biases[nm][:], dram[nm].ap())
            nc.sync.dma_start(bvbs[:], bvb.ap())
            nc.sync.dma_start(bkvbs[:], bkvb.ap())

        kgts = persist.tile([128, 4, 2, E], F8, tag="kgts")
        bvbs = persist.tile([128, D], F32, tag="bvbs")
        bkvbs = persist.tile([128, D], F32, tag="bkvbs")
        onesv = persist.tile([128, NMC, 4, 1], F8, tag="onesv")
        nc.vector.memset(onesv[:], 1.0)
        ones1 = persist.tile([1, 64], BF16, tag="ones1")
        nc.vector.memset(ones1[:], 1.0)

        qts = persist.tile([64, H, LBLK], BF16, tag="qts")
        outTs = persist.tile([128, 4, 2, LBLK], F8, tag="outTs")
        xrs = persist.tile([128, 8, LBLK], F32, tag="xrs")

        tiles = {}

        def proj_thunks(blk):
            """Block blk's projections as (main, tail) thunk lists.

            For blk==3 the kg-k groups and the last four v chunks go to
            the tail list, interleaved into block 3's own attention as
            PE filler (keeps the HAM clock up going into out-proj).
            """
            g0 = 2 * blk
            kta = blkpool.tile([64, 4, M], BF16, tag="kta",
                               name=f"kta{blk}")
            # per-head stride padded to 68 so the DoubleRow stationary AP
            # stride (mc dim = 4*68 = 272 bytes) is 16-byte aligned
            vslab = blkpool.tile([128, NMC, 4 * 68], F8, tag="vslab",
                                 name=f"vslab{blk}")
            tiles[blk] = (kta, vslab)
            nc.vector.tensor_copy(
                vslab[:].rearrange("p mc (h c) -> p mc h c", c=68)
                [:, :, :, 64:65], onesv[:])
            wv = wpool.tile([128, 4, 2, 256], F8, tag="wv", name=f"wv{blk}")
            wkv = wpool.tile([128, 4, 2, 256], F8, tag="wv", name=f"wkv{blk}")

            def load_wv():
                nc.sync.dma_start(
                    wv[:], WvT.ap().rearrange("(t two p) d -> p t two d", p=128, two=2)
                    [:, :, :, 256 * blk:256 * blk + 256])
                nc.sync.dma_start(
                    wkv[:],
                    WkvT.ap().rearrange("(t two p) d -> p t two d", p=128, two=2)
                    [:, :, :, 256 * blk:256 * blk + 256])

            def q_item(g):
                wq = w8(wqk, WqT, g)

                def run():
                    ps = psProj.tile([128, LBLK], F32, tag="psProj",
                                     name=f"qp{g}")
                    for t4 in range(4):
                        nc.tensor.matmul(ps[:], wq[:, t4, :, :],
                                         xts4[0][:, t4, :, :],
                                         start=(t4 == 0), stop=(t4 == 3),
                                         perf_mode=DR)
                    nc.vector.tensor_scalar(
                        qts[:, 2 * g, :], ps[0:64, :], 0.125,
                        biases["bq"][0:64, g:g + 1], OP.mult, OP.add)
                    nc.vector.tensor_scalar(
                        qts[:, 2 * g + 1, :], ps[64:128, :], 0.125,
                        biases["bq"][64:128, g:g + 1], OP.mult, OP.add)
                return run

            def k_item(gi, g, wk, lc):
                def run():
                    ps = psProj.tile([128, 512], F32, tag="psProj",
                                     name=f"kp{g}_{lc}")
                    for t4 in range(4):
                        nc.tensor.matmul(ps[:], wk[:, t4, :, :],
                                         xts4[lc][:, t4, :, :],
                                         start=(t4 == 0), stop=(t4 == 3),
                                         perf_mode=DR)
                    sl = slice(lc * 512, (lc + 1) * 512)
                    nc.vector.tensor_scalar_add(
                        kta[:, 2 * gi, sl], ps[0:64, :],
                        biases["bk"][0:64, g:g + 1])
                    nc.vector.tensor_scalar_add(
                        kta[:, 2 * gi + 1, sl], ps[64:128, :],
                        biases["bk"][64:128, g:g + 1])
                return run

            def kg_item(gi, g):
                def run():
                    wkk = w8(wqk, WkkT, g)
                    ps = psProj.tile([128, E], F32, tag="psProj",
                                     name=f"kkp{g}")
                    for t4 in range(4):
                        nc.tensor.matmul(ps[:], wkk[:, t4, :, :],
                                         kgts[:, t4, :, :],
                                         start=(t4 == 0), stop=(t4 == 3),
                                         perf_mode=DR)
                    nc.vector.tensor_scalar_add(
                        kta[:, 2 * gi, L:M], ps[0:64, :],
                        biases["bkk"][0:64, g:g + 1])
                    nc.vector.tensor_scalar_add(
                        kta[:, 2 * gi + 1, L:M], ps[64:128, :],
                        biases["bkk"][64:128, g:g + 1])
                return run

            dlo = 256 * blk

            def v_item(mc):
                def run():
                    ps = psProj.tile([128, 256], F32, tag="psProj",
                                     name=f"vp{blk}_{mc}")
                    for t4 in range(4):
                        if mc < 16:
                            lhsT = xts4[mc // 4][
                                :, t4, :, (mc % 4) * 128:(mc % 4 + 1) * 128]
                        else:
                            lhsT = kgts[:, t4, :,
                                        (mc - 16) * 128:(mc - 15) * 128]
                        nc.tensor.matmul(ps[:], lhsT,
                                         (wv if mc < 16 else wkv)[:, t4, :, :],
                                         start=(t4 == 0), stop=(t4 == 3),
                                         perf_mode=DR)
                    bb = bvbs if mc < 16 else bkvbs
                    nc.vector.tensor_add(
                        vslab[:, mc, :].rearrange("p (h c) -> p h c", c=68)
                        [:, :, 0:64],
                        ps[:].rearrange("p (h c) -> p h c", c=64),
                        bb[:, dlo:dlo + 256].rearrange("p (h c) -> p h c",
                                                       c=64))
                return run

            main, tail = [], []
            for gi, g in enumerate((g0, g0 + 1)):
                main.append(q_item(g))
            for gi, g in enumerate((g0, g0 + 1)):
                wk = w8(wqk, WkT, g)
                for lc in range(4):
                    main.append(k_item(gi, g, wk, lc))
                (tail if blk == 3 else main).append(kg_item(gi, g))
            load_wv()
            nv_main = 14 if blk == 3 else NMC
            for mc in range(nv_main):
                main.append(v_item(mc))
            for mc in range(nv_main, NMC):
                tail.append(v_item(mc))
            # tail order is [kg-k g0, kg-k g1, v14..v17]: kta chunks 16,17
            # land before the vslab chunks that are consumed at grp 8.
            # nvq = items that must precede any attention on this block
            # (q, k, kg); the rest are v items consumable just-in-time.
            nvq = len(main) - nv_main
            return iter(main[:nvq]), iter(main[nvq:]), iter(tail)

        def run_items(it):
            for thunk in it:
                thunk()

        # pipeline fill: only block 0's q/k/kg run un-overlapped; its v
        # items are interleaved into block 0's attention (2 per slot, one
        # slot ahead of the attn@v matmul that consumes them)
        pre0, vv0, t0 = proj_thunks(0)
        load_bulk()
        run_items(pre0)
        run_items(t0)

        tail3 = None
        vv = vv0
        for blk in range(4):
            kta, vslab = tiles[blk]
            if blk < 3:
                npre, nv, ntail = proj_thunks(blk + 1)
                nxt = __import__("itertools").chain(npre, nv)
                if blk == 2:
                    tail3 = ntail
                else:
                    nxt = __import__("itertools").chain(nxt, ntail)
            else:
                nxt = tail3
                nc.sync.dma_start(xrs[:], xres.ap())

            for hh in range(4):
                h = 4 * blk + hh
                avp = psAv.tile([65, LBLK], F32, tag="psAv", name=f"av{h}")
                ets = []

                def emit_av(grp):
                    et = ets[grp]
                    nc.tensor.matmul(
                        avp[:],
                        vslab[:, 2 * grp:2 * grp + 2, hh * 68:hh * 68 + 65],
                        et[:].rearrange("p (two n) -> p two n", two=2),
                        start=(grp == 0), stop=(grp == NMC // 2 - 1),
                        perf_mode=DR)

                for grp in range(NMC // 2):
                    for _ in range(2):
                        t = next(vv, None)
                        if t is not None:
                            t()
                    slab = psSlab.tile([128, 1024], F32, tag="psSlab",
                                       name=f"sl{h}_{grp}")
                    for c in range(2):
                        mc = 2 * grp + c
                        nc.tensor.matmul(
                            slab[:, c * 512:(c + 1) * 512],
                            kta[:, hh, mc * 128:(mc + 1) * 128],
                            qts[:, h, :], start=True, stop=True)
                    et = epool.tile([128, 1024], F8, tag="et",
                                    name=f"et{h}_{grp}")
                    nc.scalar.activation(et[:], slab[:], AF.Exp)
                    ets.append(et)
                    if grp > 0:
                        emit_av(grp - 1)
                    t = next(vv, None)
                    if t is None:
                        t = next(nxt, None)
                    if t is not None:
                        t()
                emit_av(NMC // 2 - 1)

                # numerator copy on DVE (PSUM partitions 0:64 works); the
                # denominator row stages through ACT -- a DVE read at PSUM
                # partition 64 silently returns garbage on HW.
                avs = spool.tile([64, LBLK], F32, tag="avs", name=f"avs{h}")
                nc.vector.tensor_copy(avs[:], avp[0:64, :])
                ssum = spool.tile([1, LBLK], F32, tag="ssum", name=f"ssum{h}")
                nc.scalar.activation(ssum[:], avp[64:65, :], AF.Identity)
                rec = spool.tile([1, LBLK], F32, tag="rec", name=f"rec{h}")
                nc.vector.reciprocal_approx_fast(rec[:], ssum[:])
                recr = spool.tile([1, LBLK], BF16, tag="recr", name=f"recr{h}")
                nc.vector.tensor_copy(recr[:], rec[:])
                rp = psProj.tile([64, LBLK], F32, tag="psProj", name=f"rp{h}")
                nc.tensor.matmul(rp[:], ones1[:], recr[:], start=True,
                                 stop=True)
                nc.vector.tensor_mul(
                    outTs[(h % 2) * 64:(h % 2) * 64 + 64,
                          h // 4, (h // 2) % 2, :], avs[:], rp[:])
                t = next(nxt, None)
                if t is not None:
                    t()

            run_items(nxt)
            vv = iter(())

        # ---- out-proj + gate + residual (weights one g ahead) ----
        wpairs = [(w8(wog, WoT, 0, tag="wog"), w8(wog, WgT, 0, tag="wog"))]
        for g in range(8):
            if g < 7:
                wpairs.append((w8(wog, WoT, g + 1, tag="wog"),
                               w8(wog, WgT, g + 1, tag="wog")))
            wo, wg = wpairs[g]
            pp = psProj.tile([128, LBLK], F32, tag="psProj", name=f"pp{g}")
            for t4 in range(4):
                nc.tensor.matmul(pp[:], wo[:, t4, :, :], outTs[:, t4, :, :],
                                 start=(t4 == 0), stop=(t4 == 3), perf_mode=DR)
            gp = psProj.tile([128, LBLK], F32, tag="psProj", name=f"gp{g}")
            for t4 in range(4):
                nc.tensor.matmul(gp[:], wg[:, t4, :, :], outTs[:, t4, :, :],
                                 start=(t4 == 0), stop=(t4 == 3), perf_mode=DR)
            gt = spool.tile([128, LBLK], F32, tag="gt", name=f"gt{g}")
            nc.scalar.activation(gt[:], gp[:], AF.Sigmoid,
                                 bias=biases["bge"][:, g:g + 1])
            d1 = spool.tile([128, LBLK], F32, tag="fin", name=f"d1{g}")
            nc.vector.scalar_tensor_tensor(
                d1[:], pp[:], biases["bo"][:, g:g + 1], xrs[:, g, :],
                OP.add, OP.subtract)
            d2 = spool.tile([128, LBLK], F32, tag="fin", name=f"d2{g}")
            nc.vector.tensor_mul(d2[:], d1[:], gt[:])
            fo = spool.tile([128, LBLK], BF16, tag="fin", name=f"fo{g}")
            nc.vector.tensor_add(fo[:], d2[:], xrs[:, g, :])
            nc.sync.dma_start(OUTT.ap()[g * 128:(g + 1) * 128, :], fo[:])

    nc.compile()
    return nc


def kernel(x, kg_embeds, Wq, bq, Wk, bk, Wv, bv, Wkk, bkk, Wkv, bkv,
           Wo, bo, Wg, bg):
    import ml_dtypes
    bf16 = ml_dtypes.bfloat16
    f8 = mybir.dt.np(F8)

    x = np.asarray(x, np.float32)
    kg_embeds = np.asarray(kg_embeds, np.float32)
    ws = {k: np.asarray(v, np.float32) for k, v in dict(
        Wq=Wq, bq=bq, Wk=Wk, bk=bk, Wv=Wv, bv=bv, Wkk=Wkk, bkk=bkk,
        Wkv=Wkv, bkv=bkv, Wo=Wo, bo=bo, Wg=Wg, bg=bg).items()}

    if "nc" not in _CACHE:
        _CACHE["nc"] = _build()
    nc = _CACHE["nc"]

    def col8(v):
        return np.ascontiguousarray(v.reshape(8, 128).T)

    shared = {
        "WqT": np.ascontiguousarray(ws["Wq"].T.astype(f8)),
        "WkT": np.ascontiguousarray(ws["Wk"].T.astype(f8)),
        "WkkT": np.ascontiguousarray(ws["Wkk"].T.astype(f8)),
        "WvT": np.ascontiguousarray(ws["Wv"].T.astype(f8)),
        "WkvT": np.ascontiguousarray(ws["Wkv"].T.astype(f8)),
        "WoT": np.ascontiguousarray(ws["Wo"].T.astype(f8)),
        "WgT": np.ascontiguousarray(ws["Wg"][:, :D].T.astype(f8)),
        "bq": col8(ws["bq"] * 0.125),
        "bk": col8(ws["bk"]),
        "bkk": col8(ws["bkk"]),
        "bo": col8(ws["bo"]),
        "bvb": np.ascontiguousarray(np.tile(ws["bv"], (128, 1))),
        "bkvb": np.ascontiguousarray(np.tile(ws["bkv"], (128, 1))),
    }

    in_maps = []
    for c in range(N_CORES):
        b, j = divmod(c, 4)
        xb = np.ascontiguousarray(np.roll(x[b].T, -j * LBLK, axis=1))
        kgm = kg_embeds[b].mean(axis=0)
        bge = ws["bg"] + ws["Wg"][:, D:] @ kgm
        m = dict(shared)
        m["xT"] = xb.astype(f8)
        m["xres"] = np.ascontiguousarray(
            xb[:, :LBLK].reshape(8, 128, LBLK).transpose(1, 0, 2))
        m["kgT"] = np.ascontiguousarray(kg_embeds[b].T.astype(f8))
        m["bge"] = col8(bge)
        in_maps.append(m)

    _CACHE["in_maps"] = in_maps
    res = run_bass_kernel_spmd(nc, in_maps, core_ids=list(range(N_CORES)))
    out = np.empty((B, L, D), np.float32)
    for c in range(N_CORES):
        b, j = divmod(c, 4)
        out[b, j * LBLK:(j + 1) * LBLK, :] = \
            res.results[c]["OUTT"].T.astype(np.float32)
    return out
